# revision 16
# baseline (speedup 1.0000x reference)
"""BoundingBoxRegression kernel for 8 Trainium2 NeuronCores.

Strategy: pure data parallel, one batch sample per core. Inside each core:
  - x [C,H,W] resident in SBUF as [128, C*NB, W] tiles (row r = c*H + h).
  - Phase 1: per-tile maxes -> transpose chain -> sigmoid -> per-channel
    filter thresholds Mb[128, C].
  - Per h-band (128 rows): sigmoid(x) on ACT; filter f via custom DVE select
    op (fused sum accumulator); H-conv via banded-matrix matmuls on PE
    (fp32r); W-conv via PE transposes + banded matmuls; sigmoid(conv+bias)
    on ACT from PSUM; weighted-sum stats via tensor_tensor_reduce / gpsimd
    stt / ACT Square accumulators; channel-max via gpsimd max chains;
    masked outputs via custom fused select DVE op (in-place), DMA out.
  - Tiny cross-partition reductions + bbox decode on host from stats.
"""

import numpy as np

try:
    import concourse.bass as bass
except ImportError:  # pragma: no cover
    import sys

    for _p in ("/opt/trn_rl_repo", "/root/.axon_site/_ro/trn_rl_repo"):
        if _p not in sys.path:
            sys.path.insert(0, _p)
    import concourse.bass as bass

from concourse import mybir
from concourse import tile
from concourse.bass_utils import run_bass_kernel_spmd
import concourse.dve_ops as dve_ops
from concourse.dve_ops import DveOp, has_src1
from concourse.dve_spec import Spec, Src0, Src1, C0, C1, Zero, select, lower
from concourse.dve_uop import DveOpSpec, AluOp as UAluOp
from concourse.dve_table_gen import dve_ver_for

F32 = mybir.dt.float32
F32R = mybir.dt.float32r
AF = mybir.ActivationFunctionType
ALU = mybir.AluOpType
AX = mybir.AxisListType

SCALE = 0.01
KSIZE = 11
PAD = 5
N_CORES = 8

# ---------------------------------------------------------------- custom ops


def _register_dve_op(name, spec):
    for op in dve_ops.OPS:
        if op.name == name:
            return op
    opcode = dve_ops._CUSTOM_DVE_ROW_BASE + len(dve_ops.OPS)
    assert opcode < 0x20
    shas = {}
    for ver in ("v3", "v4"):
        try:
            uops = lower(spec, ver=ver)
            shas[ver] = DveOpSpec(
                name=name, opcode=opcode, uops=uops, rd1_en=has_src1(spec)
            ).sha(ver)
        except Exception:
            pass
    op = DveOp(name, spec, subdim=False, uops_sha=shas)
    dve_ops.OPS.append(op)
    dve_ops._SUB_OPCODE_FOR_NAME[name] = opcode
    dve_ops.CUSTOM_DVE_SPECS[name] = spec
    return op


def _filter_ref(in0, in1, s0, s1, imm2):
    b = (in0 * s1 + np.where(in0 > s0, in0, 0.0)).astype(np.float32)
    return b, b.reshape(b.shape[0], -1).sum(axis=-1, keepdims=True)


# out = s*C1 + (s if s > C0 else 0); accum_out = sum(out)
FILTER_SEL = _register_dve_op(
    "FILTER_SEL_ANT",
    Spec(
        body=Src0 * C1 + select(Src0 > C0, Src0, Zero),
        accum=UAluOp.ADD,
        reference=_filter_ref,
    ),
)

def _mcr_ref(ge):
    def _r(in0, in1, s0, s1, imm2):
        in1 = np.asarray(in1).reshape(np.asarray(in0).shape)
        keep = (in0 >= in1) if ge else (in0 <= in1)
        return np.where(keep, in0 * s0 + s1, 0.0).astype(np.float32)

    return _r


# out = (g*C0 + C1) if g >= m else 0   (monotone-increasing affine)
MCR_SEL_GE = _register_dve_op(
    "MCR_SEL_GE_ANT",
    Spec(body=select(Src0 >= Src1, Src0 * C0 + C1, Zero), reference=_mcr_ref(True)),
)

# out = (g*C0 + C1) if g <= m else 0   (monotone-decreasing affine; m = min)
MCR_SEL_LE = _register_dve_op(
    "MCR_SEL_LE_ANT",
    Spec(body=select(Src1 >= Src0, Src0 * C0 + C1, Zero), reference=_mcr_ref(False)),
)


# ---------------------------------------------------------------- host consts


def _band_matrix(kern, n):
    """M[i, j] = kern[i - j + PAD] for |i - j| <= PAD else 0 (n x n)."""
    m = np.zeros((n, n), np.float64)
    for k in range(KSIZE):
        d = k - PAD  # i - j offset of this tap
        diag = np.full(n - abs(d), kern[k], np.float64)
        # np.diag(v, k) places v at (i, i+k), i.e. i - j = -k -> k = -d.
        m += np.diag(diag, k=-d)
    return m


def _host_consts(meta):
    C, H, W = meta["C"], meta["H"], meta["W"]
    NB, NW = H // 128, W // 128
    a_full = _band_matrix(np.asarray(meta["h_kernel"], np.float64), H)
    pairs = [(i, j) for j in range(NB) for i in range(NB) if abs(i - j) <= 1]
    ablk = np.stack(
        [a_full[i * 128 : (i + 1) * 128, j * 128 : (j + 1) * 128] for (i, j) in pairs]
    ).astype(np.float32)
    bmat = _band_matrix(np.asarray(meta["w_kernel"], np.float64), W).astype(np.float32)
    eye = np.eye(128, dtype=np.float32)
    colv = np.broadcast_to(np.arange(W, dtype=np.float32), (128, W)).copy()
    return {
        "pairs": pairs,
        "ablk": ablk.reshape(len(pairs) * 128, 128),
        "bblk": bmat,  # [W, W]
        "eye": eye,
        "colv": colv,
    }


# ---------------------------------------------------------------- the program


def build_program(meta):
    """Build the single-core Bass program (SPMD across cores).

    meta: C, H, W, w_bias, h_bias, sk, sb, w_kernel, h_kernel.
    Returns (nc, consts_dict).
    """
    C, H, W = meta["C"], meta["H"], meta["W"]
    NB, NW = H // 128, W // 128
    NT = C * NB
    consts = _host_consts(meta)
    pairs = consts["pairs"]
    npairs = len(pairs)
    w_bias = float(meta["w_bias"])
    h_bias = float(meta["h_bias"])
    sk = float(meta["sk"])
    sb = float(meta["sb"])
    skW = sk * W
    skH = sk * H
    sel_op = MCR_SEL_GE if sk >= 0 else MCR_SEL_LE
    tree_alu = ALU.max if sk >= 0 else ALU.min

    from concourse import bacc

    nc = bacc.Bacc("TRN2", debug=False)

    xin = nc.dram_tensor("xin", [C * H, W], F32, kind="ExternalInput").ap()
    a_d = nc.dram_tensor("ablk", [npairs * 128, 128], F32, kind="ExternalInput").ap()
    b_d = nc.dram_tensor("bblk", [W, W], F32, kind="ExternalInput").ap()
    eye_d = nc.dram_tensor("eye", [128, 128], F32, kind="ExternalInput").ap()
    colv_d = nc.dram_tensor("colv", [128, W], F32, kind="ExternalInput").ap()

    mcr_d = nc.dram_tensor("mcr", [3 * C * H, W], F32, kind="ExternalOutput").ap()
    statsv_d = nc.dram_tensor("statsv", [128, 3 * NT], F32, kind="ExternalOutput").ap()
    statsg_d = nc.dram_tensor("statsg", [128, NT], F32, kind="ExternalOutput").ap()
    statsa_d = nc.dram_tensor("statsa", [128, NT], F32, kind="ExternalOutput").ap()

    xin_v = xin.rearrange("(c j p) w -> j p c w", c=C, j=NB, p=128)
    mcr_v = mcr_d.rearrange("(g c j p) w -> g j p c w", g=3, c=C, j=NB, p=128)

    with tile.TileContext(nc) as tc:
        with (
            tc.tile_pool(name="xpool", bufs=1) as xpool,
            tc.tile_pool(name="cpool", bufs=1) as cpool,
            tc.tile_pool(name="spool", bufs=3) as spool,
            tc.tile_pool(name="fpool", bufs=1) as fpool,
            tc.tile_pool(name="gwpool", bufs=1) as gwpool,
            tc.tile_pool(name="ghpool", bufs=1) as ghpool,
            tc.tile_pool(name="mpool", bufs=2) as mpool,
            tc.tile_pool(name="xtpool", bufs=2) as xtpool,
            tc.tile_pool(name="stpool", bufs=1) as stpool,
            tc.tile_pool(name="jpool", bufs=1) as jpool,
            tc.tile_pool(name="xt_ps", bufs=1, space="PSUM") as xt_ps_pool,
            tc.tile_pool(name="conv_ps", bufs=2, space="PSUM") as conv_ps_pool,
            tc.tile_pool(name="small_ps", bufs=1, space="PSUM") as small_ps_pool,
        ):
            # ---------------- constants
            a_sb = cpool.tile([128, npairs, 128], F32)
            nc.sync.dma_start(a_sb[:], a_d.rearrange("(k p) m -> p k m", p=128))
            b_sb = cpool.tile([128, NW, W], F32)
            nc.sync.dma_start(b_sb[:], b_d.rearrange("(k p) w -> p k w", p=128))
            eye_sb = cpool.tile([128, 128], F32)
            nc.sync.dma_start(eye_sb[:], eye_d[:])
            colv_sb = cpool.tile([128, W], F32)
            nc.sync.dma_start(colv_sb[:], colv_d[:])

            # ---------------- x load (per band) + per-tile maxes
            x_b = []
            mstats = stpool.tile([128, NT], F32)
            for j in range(NB):
                xb = xpool.tile([128, C, W], F32, tag=f"x{j}")
                nc.sync.dma_start(xb[:], xin_v[j])
                x_b.append(xb)
            for j in range(NB):
                for c in range(C):
                    t = c * NB + j
                    nc.vector.tensor_reduce(
                        mstats[:, t : t + 1], x_b[j][:, c, :], AX.X, ALU.max
                    )

            # ---------------- Mb chain: per-channel sigmoid(max) - 0.005
            mt_ps = small_ps_pool.tile([NT, 128], F32, tag="sps")
            nc.tensor.transpose(mt_ps[:], mstats[:, :NT], eye_sb[:])
            mrow = stpool.tile([NT, 1], F32)
            nc.vector.tensor_reduce(mrow[:], mt_ps[:], AX.X, ALU.max)
            mrow_ps = small_ps_pool.tile([1, NT], F32, tag="sps")
            nc.tensor.transpose(mrow_ps[:], mrow[:], eye_sb[:NT, :NT])
            mrow_t = stpool.tile([1, NT], F32)
            nc.vector.tensor_copy(mrow_t[:], mrow_ps[:])
            mmax = stpool.tile([1, C], F32)
            nc.vector.tensor_reduce(
                mmax[:], mrow_t[:].rearrange("a (c j) -> a c j", c=C), AX.X, ALU.max
            )
            msig = stpool.tile([1, C], F32)
            nc.scalar.activation(msig[:], mmax[:], AF.Sigmoid)
            thr = stpool.tile([1, C], F32)
            nc.vector.tensor_scalar(thr[:], msig[:], -0.005, None, ALU.add)
            mb = stpool.tile([128, C], F32)
            nc.gpsimd.partition_broadcast(mb[:], thr[:])

            # bias tiles for the conv sigmoids
            wb_t = stpool.tile([128, 1], F32)
            nc.gpsimd.memset(wb_t[:], w_bias)
            hb_t = stpool.tile([128, 1], F32)
            nc.gpsimd.memset(hb_t[:], h_bias)

            # ---------------- stats accumulators
            statsv = stpool.tile([128, 3 * NT], F32)
            statsg = stpool.tile([128, NT], F32)
            statsa = stpool.tile([128, NT], F32)
            junkv = jpool.tile([128, W], F32, tag="jv")
            junkg = jpool.tile([128, W], F32, tag="jg")
            junka = jpool.tile([128, W], F32, tag="ja")

            # ---------------- band loop
            for j in range(NB):
                xb = x_b[j]
                f_band = fpool.tile([128, C, W], F32, tag="f")
                gw_band = gwpool.tile([128, C, W], F32, tag="gw")
                gh_band = ghpool.tile([128, C, W], F32, tag="gh")

                # sigmoid(x) in 4-channel batches
                s_tiles = {}
                for cb in range(0, C, 4):
                    cn = min(4, C - cb)
                    st = spool.tile([128, cn, W], F32, tag="s")
                    nc.scalar.activation(st[:], xb[:, cb : cb + cn, :], AF.Sigmoid)
                    s_tiles[cb] = st

                hp = [(i, idx) for idx, (i, jj) in enumerate(pairs) if jj == j]
                for c in range(C):
                    t = c * NB + j
                    s_ap = s_tiles[(c // 4) * 4][:, c % 4, :]
                    # filter: f = s*SCALE + (s if s > thr else 0); accum sum
                    nc.vector._custom_dve(
                        FILTER_SEL,
                        out=f_band[:, c, :],
                        in0=s_ap,
                        s0=mb[:, c : c + 1],
                        s1=SCALE,
                        accum_out=statsv[:, t : t + 1],
                    )

                for p0 in range(0, C, 2):
                    cs = [p0] if C == 1 else [p0, p0 + 1]
                    ncr = len(cs)
                    xt_ps = xt_ps_pool.tile([128, ncr * W], F32, tag="xt")
                    xt_sb = xtpool.tile([128, ncr * W], F32, tag="xts")
                    wc_ps = conv_ps_pool.tile([128, ncr, 512], F32, tag="conv")
                    hc_ps = conv_ps_pool.tile([128, ncr, 512], F32, tag="conv")
                    for half, c in enumerate(cs):
                        for k in range(NW):
                            nc.tensor.transpose(
                                xt_ps[
                                    :, half * W + k * 128 : half * W + (k + 1) * 128
                                ],
                                xb[:, c, k * 128 : (k + 1) * 128],
                                eye_sb[:],
                            )
                    # drain transposes PSUM -> SBUF, alternating engines
                    if (p0 // 2) % 2 == 0:
                        nc.vector.tensor_copy(xt_sb[:], xt_ps[:])
                    else:
                        nc.scalar.copy(xt_sb[:], xt_ps[:])
                    for half, c in enumerate(cs):
                        wslc = wc_ps[:, half, :W]
                        for k in range(NW):
                            nc.tensor.matmul(
                                wslc,
                                xt_sb[
                                    :, half * W + k * 128 : half * W + (k + 1) * 128
                                ],
                                b_sb[:, k, :],
                                start=(k == 0),
                                stop=(k == NW - 1),
                            )
                        hslc = hc_ps[:, half, :W]
                        for n, (i, idx) in enumerate(hp):
                            nc.tensor.matmul(
                                hslc,
                                a_sb[:, idx, :],
                                x_b[i][:, c, :],
                                start=(n == 0),
                                stop=(n == len(hp) - 1),
                            )
                    nc.scalar.activation(
                        gw_band[:, p0 : p0 + ncr, :],
                        wc_ps[:, :, :W],
                        AF.Sigmoid,
                        bias=wb_t[:],
                    )
                    nc.scalar.activation(
                        gh_band[:, p0 : p0 + ncr, :],
                        hc_ps[:, :, :W],
                        AF.Sigmoid,
                        bias=hb_t[:],
                    )

                for c in range(C):
                    t = c * NB + j
                    # sum f*sgw, f*sgh (DVE), f*col (gpsimd), f^2 (ACT)
                    nc.vector.scalar_tensor_tensor(
                        junkv[:],
                        f_band[:, c, :],
                        1.0,
                        gw_band[:, c, :],
                        ALU.bypass,
                        ALU.mult,
                        accum_out=statsv[:, NT + t : NT + t + 1],
                    )
                    nc.vector.scalar_tensor_tensor(
                        junkv[:],
                        f_band[:, c, :],
                        1.0,
                        gh_band[:, c, :],
                        ALU.bypass,
                        ALU.mult,
                        accum_out=statsv[:, 2 * NT + t : 2 * NT + t + 1],
                    )
                    nc.vector.scalar_tensor_tensor(
                        junkg[:],
                        f_band[:, c, :],
                        1.0,
                        colv_sb[:],
                        ALU.bypass,
                        ALU.mult,
                        accum_out=statsg[:, t : t + 1],
                    )
                    nc.scalar.activation(
                        junka[:],
                        f_band[:, c, :],
                        AF.Square,
                        accum_out=statsa[:, t : t + 1],
                    )

                # channel-max (or min) chains on gpsimd, then fused selects
                for g, (src, a0, a1, alu) in enumerate(
                    [
                        (f_band, 1.0, 0.0, ALU.max),
                        (gw_band, skW, sb, tree_alu),
                        (gh_band, skH, sb, tree_alu),
                    ]
                ):
                    m_g = mpool.tile([128, W], F32, tag="m")
                    nc.vector.tensor_reduce(
                        m_g[:], src[:].rearrange("p c w -> p w c"), AX.X, alu
                    )
                    m_bc = m_g[:].unsqueeze(1).broadcast_to((128, C, W))
                    op = MCR_SEL_GE if g == 0 else sel_op
                    nc.vector._custom_dve(
                        op,
                        out=src[:, :, :],
                        in0=src[:, :, :],
                        in1=m_bc,
                        s0=a0,
                        s1=a1,
                    )
                    nc.sync.dma_start(mcr_v[g, j], src[:, :, :])

            nc.sync.dma_start(statsv_d[:], statsv[:])
            nc.sync.dma_start(statsg_d[:], statsg[:])
            nc.sync.dma_start(statsa_d[:], statsa[:])

    nc.compile()
    return nc, consts


# ---------------------------------------------------------------- host glue

_CACHE = {}


def _get_program(meta_key, meta):
    if meta_key not in _CACHE:
        _CACHE[meta_key] = build_program(meta)
    return _CACHE[meta_key]


def _decode(meta, statsv, statsg, statsa, b_idx):
    """Per-core bbox decode from stats tensors -> [C, 6] rows."""
    C, H, W = meta["C"], meta["H"], meta["W"]
    NB = H // 128
    NT = C * NB
    sk, sb = meta["sk"], meta["sb"]
    fst = statsv[:, :NT].astype(np.float64).reshape(128, C, NB)
    wst = statsv[:, NT : 2 * NT].astype(np.float64).reshape(128, C, NB)
    hst = statsv[:, 2 * NT : 3 * NT].astype(np.float64).reshape(128, C, NB)
    cst = statsg[:, :NT].astype(np.float64).reshape(128, C, NB)
    qst = statsa[:, :NT].astype(np.float64).reshape(128, C, NB)

    F = fst.sum(axis=(0, 2))  # [C]
    hval = (np.arange(NB)[None, :] * 128 + np.arange(128)[:, None]).astype(np.float64)
    Sfr = np.einsum("pcj,pj->c", fst, hval)
    Sfw = wst.sum(axis=(0, 2))
    Sfh = hst.sum(axis=(0, 2))
    Sfc = cst.sum(axis=(0, 2))
    Sff = qst.sum(axis=(0, 2))

    score = Sff / F
    wb = (sk * W * Sfw + sb * F) / F
    hb = (sk * H * Sfh + sb * F) / F
    cb = Sfc / F
    rb = Sfr / F
    x1 = cb - wb / 2
    y1 = rb - hb / 2
    x2 = cb + wb / 2
    y2 = rb + hb / 2
    out = np.stack(
        [np.full(C, b_idx, np.float64), x1, y1, x2, y2, score], axis=-1
    ).astype(np.float32)
    return out


def _run(inputs, trace=False, **run_kwargs):
    x = np.ascontiguousarray(np.asarray(inputs["x"], np.float32))
    B, C, H, W = x.shape
    meta = {
        "C": C,
        "H": H,
        "W": W,
        "w_kernel": np.asarray(inputs["w_kernel"], np.float32),
        "h_kernel": np.asarray(inputs["h_kernel"], np.float32),
        "w_bias": float(np.asarray(inputs["w_bias"]).reshape(-1)[0]),
        "h_bias": float(np.asarray(inputs["h_bias"]).reshape(-1)[0]),
        "sk": float(np.asarray(inputs["s_kernel"]).reshape(-1)[0]),
        "sb": float(np.asarray(inputs["s_bias"]).reshape(-1)[0]),
    }
    meta_key = (
        C,
        H,
        W,
        meta["w_kernel"].tobytes(),
        meta["h_kernel"].tobytes(),
        meta["w_bias"],
        meta["h_bias"],
        meta["sk"],
        meta["sb"],
    )
    nc, consts = _get_program(meta_key, meta)

    in_maps = []
    for b in range(B):
        in_maps.append(
            {
                "xin": np.ascontiguousarray(x[b].reshape(C * H, W)),
                "ablk": consts["ablk"],
                "bblk": consts["bblk"],
                "eye": consts["eye"],
                "colv": consts["colv"],
            }
        )
    res = run_bass_kernel_spmd(
        nc, in_maps, core_ids=list(range(B)), trace=trace, **run_kwargs
    )

    mcr = np.stack(
        [res.results[b]["mcr"].reshape(3 * C, H, W) for b in range(B)]
    ).astype(np.float32)
    rows = np.concatenate(
        [
            _decode(
                meta,
                res.results[b]["statsv"],
                res.results[b]["statsg"],
                res.results[b]["statsa"],
                b,
            )
            for b in range(B)
        ]
    ).astype(np.float32)
    return (mcr, rows), res


def kernel(**inputs):
    out, _ = _run(inputs, trace=False)
    return out
